# revision 5
# baseline (speedup 1.0000x reference)
"""Tensor-parallel GQA attention prefill for 8 TRN2 NeuronCores.

Sharding: each core owns 4 query heads + 1 kv head (column-shard of
wq/wk/wv by head) and a 512-row slice of wo's input dim (row-shard).
Each core computes a partial output projection over its local heads;
the host sums the 8 partials (equivalent to the all-reduce in the
sharding hint) and transposes back to [b, s, d].

Device math (per core), all layouts feature-on-partitions:
  qT/kT/vT = W^T-tile.T @ xT-tile accumulated over 32 d-tiles (fp32r)
  RoPE applied in "rotate_half" form: weight rows are pre-permuted on
  the host (even features first, then odd) so the pair (2i, 2i+1)
  becomes (i, i+64) and the cross-partition shuffle is two 64-partition
  copies instead of a stride-2 partition gather.
  scores[tq,tk] per (b,h) via matmul over head_dim, +mask, softmax on
  the free axis, PE-transpose of probs, PV matmul, then the wo matmul
  over local features only (partial sums, bf16).
"""

import math
from contextlib import ExitStack

import ml_dtypes
import numpy as np

import concourse.bass as bass
import concourse.tile as tile
from concourse import bacc, mybir
from concourse.bass_utils import run_bass_kernel_spmd

DIM = 4096
N_HEADS = 32
HEAD_DIM = 128
N_KV_HEADS = 8
BSZ = 4
SEQLEN = 128
T = BSZ * SEQLEN  # 512 tokens
NCORES = 8
HQ = N_HEADS // NCORES  # 4 query heads per core
EQ = HQ * HEAD_DIM  # 512 local q features
ND = DIM // 128  # 32 contraction tiles
SCALE = 1.0 / math.sqrt(HEAD_DIM)

F32 = mybir.dt.float32
F32R = mybir.dt.float32r
BF16 = mybir.dt.bfloat16
AX = mybir.AxisListType
ACTF = mybir.ActivationFunctionType
PSUM = bass.MemorySpace.PSUM

_STATE: dict = {}
LAST_RESULT = None


def _install_ntff_hook():
    """Register the axon NTFF profile hook if the image lacks antenv.axon_hooks.

    Lets run_bass_kernel_spmd(trace=True) return exec_time_ns + perfetto
    under axon. Best-effort: any failure leaves tracing disabled but the
    kernel still runs.
    """
    import os
    import sys
    import types

    try:
        import antenv.axon_hooks  # noqa: F401

        return
    except ImportError:
        pass
    try:
        import antenv
        from trn_agent_boot.trn_boot import _ntff_profile_via_ctypes

        mod = types.ModuleType("antenv.axon_hooks")
        holder = {"hook": None}
        mod.set_axon_ntff_profile_hook = lambda h: holder.__setitem__("hook", h)
        mod.get_axon_ntff_profile_hook = lambda: holder["hook"]
        sys.modules["antenv.axon_hooks"] = mod
        antenv.axon_hooks = mod
        so = "/opt/axon/libaxon_pjrt.so"
        if os.path.exists(so):
            hook = _ntff_profile_via_ctypes(so)
            if hook is not None:
                mod.set_axon_ntff_profile_hook(hook)
    except Exception:
        pass


_install_ntff_hook()


def _build_nc():
    nc = bacc.Bacc(
        "TRN2",
        target_bir_lowering=False,
        debug=False,
        enable_asserts=False,
        num_devices=NCORES,
    )
    xT = nc.dram_tensor("xT", [DIM, T], F32R, kind="ExternalInput").ap()
    wqT = nc.dram_tensor("wqT", [DIM, EQ], F32R, kind="ExternalInput").ap()
    wkT = nc.dram_tensor("wkT", [DIM, HEAD_DIM], F32R, kind="ExternalInput").ap()
    wvT = nc.dram_tensor("wvT", [DIM, HEAD_DIM], F32R, kind="ExternalInput").ap()
    woT = nc.dram_tensor("woT", [EQ, DIM], BF16, kind="ExternalInput").ap()
    mask4 = nc.dram_tensor("mask4", [128, T], F32, kind="ExternalInput").ap()
    cq = nc.dram_tensor("cq", [128, T], F32, kind="ExternalInput").ap()
    sq = nc.dram_tensor("sq", [128, T], F32, kind="ExternalInput").ap()
    ck = nc.dram_tensor("ck", [128, T], F32, kind="ExternalInput").ap()
    sk = nc.dram_tensor("sk", [128, T], F32, kind="ExternalInput").ap()
    ident = nc.dram_tensor("ident", [128, 128], BF16, kind="ExternalInput").ap()
    yT = nc.dram_tensor("yT", [DIM, T], BF16, kind="ExternalOutput").ap()

    with tile.TileContext(nc) as tc, ExitStack() as ctx:
        const = ctx.enter_context(tc.tile_pool(name="const", bufs=1))
        xp = ctx.enter_context(tc.tile_pool(name="xp", bufs=4))
        wqp = ctx.enter_context(tc.tile_pool(name="wqp", bufs=4))
        wkvp = ctx.enter_context(tc.tile_pool(name="wkvp", bufs=4))
        rt = ctx.enter_context(tc.tile_pool(name="rt", bufs=3))
        sm = ctx.enter_context(tc.tile_pool(name="sm", bufs=3))
        yp = ctx.enter_context(tc.tile_pool(name="yp", bufs=4))
        ps = ctx.enter_context(tc.tile_pool(name="ps", bufs=8, space=PSUM))

        # resident constants / activations
        mask_sb = const.tile([128, T], F32, tag="mask")
        nc.sync.dma_start(mask_sb[:], mask4)
        cq_sb = const.tile([128, T], F32, tag="cq")
        nc.sync.dma_start(cq_sb[:], cq)
        sq_sb = const.tile([128, T], F32, tag="sq")
        nc.sync.dma_start(sq_sb[:], sq)
        ck_sb = const.tile([128, T], F32, tag="ck")
        nc.sync.dma_start(ck_sb[:], ck)
        sk_sb = const.tile([128, T], F32, tag="sk")
        nc.sync.dma_start(sk_sb[:], sk)
        ident_sb = const.tile([128, 128], BF16, tag="ident")
        nc.sync.dma_start(ident_sb[:], ident)
        wo_sb = const.tile([128, HQ * DIM], BF16, tag="wo")
        nc.sync.dma_start(
            wo_sb[:].rearrange("p (j c) -> p j c", j=HQ),
            woT.rearrange("(j p) c -> p j c", p=128),
        )

        qT_sb = const.tile([128, HQ * T], F32, tag="qT")
        kT_sb = const.tile([128, T], F32, tag="kT")
        vT_sb = const.tile([128, T], BF16, tag="vT")
        v_sb = const.tile([128, BSZ * HEAD_DIM], BF16, tag="v")
        oT_sb = const.tile([128, HQ * T], BF16, tag="oT")

        # ---- projection pass: one sweep over the 32 d-tiles ----
        ps_q = [ps.tile([128, T], F32, tag="ps", name=f"ps_q{h}") for h in range(HQ)]
        ps_k = ps.tile([128, T], F32, tag="ps")
        ps_v = ps.tile([128, T], F32, tag="ps")
        for j in range(ND):
            rows = slice(j * 128, (j + 1) * 128)
            xt = xp.tile([128, T], F32R, tag="x")
            nc.sync.dma_start(xt[:], xT[rows, :])
            wqt = wqp.tile([128, EQ], F32R, tag="wq")
            nc.sync.dma_start(wqt[:], wqT[rows, :])
            wkt = wkvp.tile([128, HEAD_DIM], F32R, tag="wk")
            nc.sync.dma_start(wkt[:], wkT[rows, :])
            wvt = wkvp.tile([128, HEAD_DIM], F32R, tag="wv")
            nc.sync.dma_start(wvt[:], wvT[rows, :])
            st, sp = (j == 0), (j == ND - 1)
            xr = xt[:]
            for h in range(HQ):
                nc.tensor.matmul(
                    ps_q[h][:],
                    wqt[:, h * 128 : (h + 1) * 128],
                    xr,
                    start=st,
                    stop=sp,
                )
            nc.tensor.matmul(ps_k[:], wkt[:], xr, start=st, stop=sp)
            nc.tensor.matmul(ps_v[:], wvt[:], xr, start=st, stop=sp)

        # ---- RoPE (rotate_half form thanks to host-side row permutation) ----
        def rope(dst_ap, pssrc, ctab, stab):
            swp = rt.tile([128, T], F32, tag="swp")
            nc.scalar.copy(swp[0:64, :], pssrc[64:128, :])
            nc.scalar.copy(swp[64:128, :], pssrc[0:64, :])
            prod = rt.tile([128, T], F32, tag="prod")
            nc.vector.tensor_mul(prod[:], pssrc[:], ctab)
            nc.vector.tensor_mul(swp[:], swp[:], stab)
            nc.vector.tensor_add(dst_ap, prod[:], swp[:])

        rope(kT_sb[:], ps_k[:], ck_sb[:], sk_sb[:])
        # v: evict to bf16, then transpose per batch into [tk, hd]
        nc.scalar.copy(vT_sb[:], ps_v[:])
        for b in range(BSZ):
            bs = slice(b * 128, (b + 1) * 128)
            ps_t = ps.tile([128, T], BF16, tag="ps")
            nc.tensor.transpose(ps_t[:, 0:128], vT_sb[:, bs], ident_sb[:])
            nc.vector.tensor_copy(v_sb[:, bs], ps_t[:, 0:128])
        for h in range(HQ):
            rope(qT_sb[:, h * T : (h + 1) * T], ps_q[h][:], cq_sb[:], sq_sb[:])

        # ---- attention per local head ----
        for h in range(HQ):
            ps_s = ps.tile([128, T], F32, tag="ps")
            for b in range(BSZ):
                bs = slice(b * 128, (b + 1) * 128)
                nc.tensor.matmul(
                    ps_s[:, bs],
                    qT_sb[:, h * T + b * 128 : h * T + (b + 1) * 128],
                    kT_sb[:, bs],
                    start=True,
                    stop=True,
                )
            s_sb = sm.tile([128, T], F32, tag="s")
            nc.vector.tensor_add(s_sb[:], ps_s[:], mask_sb[:])
            nmx = sm.tile([128, BSZ], F32, tag="nmx")
            den = sm.tile([128, BSZ], F32, tag="den")
            rden = sm.tile([128, BSZ], F32, tag="rden")
            p_sb = sm.tile([128, T], BF16, tag="p")
            for b in range(BSZ):
                bs = slice(b * 128, (b + 1) * 128)
                nc.vector.reduce_max(
                    nmx[:, b : b + 1], s_sb[:, bs], axis=AX.X, negate=True
                )
                nc.scalar.activation(
                    p_sb[:, bs],
                    s_sb[:, bs],
                    ACTF.Exp,
                    bias=nmx[:, b : b + 1],
                    accum_out=den[:, b : b + 1],
                )
            nc.vector.reciprocal(rden[:], den[:])
            for b in range(BSZ):
                bs = slice(b * 128, (b + 1) * 128)
                nc.vector.tensor_scalar_mul(p_sb[:, bs], p_sb[:, bs], rden[:, b : b + 1])
                ps_pt = ps.tile([128, T], BF16, tag="ps")
                nc.tensor.transpose(ps_pt[:, 0:128], p_sb[:, bs], ident_sb[:])
                pt_sb = sm.tile([128, 128], BF16, tag="pt")
                nc.scalar.copy(pt_sb[:], ps_pt[:, 0:128])
                ps_o = ps.tile([128, T], F32, tag="ps")
                nc.tensor.matmul(
                    ps_o[:, 0:128], v_sb[:, bs], pt_sb[:], start=True, stop=True
                )
                nc.vector.tensor_copy(
                    oT_sb[:, h * T + b * 128 : h * T + (b + 1) * 128], ps_o[:, 0:128]
                )

        # ---- output projection over local features (partial sums) ----
        for dt in range(ND):
            ps_y = ps.tile([128, T], F32, tag="ps")
            for j in range(HQ):
                nc.tensor.matmul(
                    ps_y[:],
                    wo_sb[:, j * DIM + dt * 128 : j * DIM + (dt + 1) * 128],
                    oT_sb[:, j * T : (j + 1) * T],
                    start=(j == 0),
                    stop=(j == HQ - 1),
                )
            y_sb = yp.tile([128, T], BF16, tag="y")
            if dt % 2 == 0:
                nc.vector.tensor_copy(y_sb[:], ps_y[:])
            else:
                nc.scalar.copy(y_sb[:], ps_y[:])
            nc.sync.dma_start(yT[dt * 128 : (dt + 1) * 128, :], y_sb[:])

    nc.compile()
    return nc


def get_nc():
    if "nc" not in _STATE:
        _STATE["nc"] = _build_nc()
    return _STATE["nc"]


def _prep_in_maps(x, wq, wk, wv, wo, freqs_cos, freqs_sin, mask):
    f32 = np.float32
    x = np.asarray(x, f32)
    wq = np.asarray(wq, f32)
    wk = np.asarray(wk, f32)
    wv = np.asarray(wv, f32)
    wo = np.asarray(wo, f32)
    fc = np.asarray(freqs_cos, f32)
    fs = np.asarray(freqs_sin, f32)
    mask = np.asarray(mask, f32)

    # even features first, then odd: (2i, 2i+1) pairs -> (i, i+64)
    perm = np.concatenate([np.arange(0, HEAD_DIM, 2), np.arange(1, HEAD_DIM, 2)])
    wqp = wq.reshape(N_HEADS, HEAD_DIM, DIM)[:, perm, :].reshape(DIM, DIM)
    wkp = wk.reshape(N_KV_HEADS, HEAD_DIM, DIM)[:, perm, :].reshape(
        N_KV_HEADS * HEAD_DIM, DIM
    )

    xT = np.ascontiguousarray(x.reshape(T, DIM).T)
    C0 = np.vstack([fc.T, fc.T])  # [128, 128]: row p -> cos[t, p % 64]
    S0 = np.vstack([-fs.T, fs.T])
    cq = np.ascontiguousarray(np.tile(C0 * SCALE, (1, BSZ)))
    sq = np.ascontiguousarray(np.tile(S0 * SCALE, (1, BSZ)))
    ck = np.ascontiguousarray(np.tile(C0, (1, BSZ)))
    sk = np.ascontiguousarray(np.tile(S0, (1, BSZ)))
    mask4 = np.ascontiguousarray(np.tile(mask[0, 0], (1, BSZ)))
    ident = np.eye(128, dtype=ml_dtypes.bfloat16)

    in_maps = []
    for c in range(NCORES):
        qrows = slice(c * EQ, (c + 1) * EQ)
        krows = slice(c * HEAD_DIM, (c + 1) * HEAD_DIM)
        in_maps.append(
            {
                "xT": xT,
                "wqT": np.ascontiguousarray(wqp[qrows, :].T),
                "wkT": np.ascontiguousarray(wkp[krows, :].T),
                "wvT": np.ascontiguousarray(wv[krows, :].T),
                "woT": np.ascontiguousarray(wo[:, qrows].T).astype(ml_dtypes.bfloat16),
                "mask4": mask4,
                "cq": cq,
                "sq": sq,
                "ck": ck,
                "sk": sk,
                "ident": ident,
            }
        )
    return in_maps


def kernel(
    x,
    wq,
    wk,
    wv,
    wo,
    cache_k,
    cache_v,
    freqs_cos,
    freqs_sin,
    mask,
    start_pos,
    *,
    trace=False,
    trace_kwargs=None,
):
    global LAST_RESULT
    sp = int(np.asarray(start_pos))
    assert sp == 0, f"kernel specialized for start_pos=0, got {sp}"

    in_maps = _prep_in_maps(x, wq, wk, wv, wo, freqs_cos, freqs_sin, mask)
    nc = get_nc()
    res = run_bass_kernel_spmd(
        nc,
        in_maps,
        core_ids=list(range(NCORES)),
        trace=trace,
        **(trace_kwargs or {}),
    )
    LAST_RESULT = res
    acc = np.zeros((DIM, T), np.float32)
    for c in range(NCORES):
        acc += res.results[c]["yT"].astype(np.float32)
    return np.ascontiguousarray(acc.T).reshape(BSZ, SEQLEN, DIM)


# revision 7
# speedup vs baseline: 1.0557x; 1.0557x over previous
"""Tensor-parallel GQA attention prefill for 8 TRN2 NeuronCores.

Sharding: each core owns 4 query heads + 1 kv head (column-shard of
wq/wk/wv by head) and a 512-row slice of wo's input dim (row-shard).
Each core computes a partial output projection over its local heads;
the host sums the 8 partials (equivalent to the all-reduce in the
sharding hint) and transposes back to [b, s, d].

Device math (per core), all layouts feature-on-partitions:
  qT/kT/vT = W^T-tile.T @ xT-tile accumulated over 32 d-tiles (fp32r)
  RoPE applied in "rotate_half" form: weight rows are pre-permuted on
  the host (even features first, then odd) so the pair (2i, 2i+1)
  becomes (i, i+64) and the cross-partition shuffle is two 64-partition
  copies instead of a stride-2 partition gather.
  scores[tq,tk] per (b,h) via matmul over head_dim, +mask, softmax on
  the free axis, PE-transpose of probs, PV matmul, then the wo matmul
  over local features only (partial sums, bf16).
"""

import math
from contextlib import ExitStack

import ml_dtypes
import numpy as np

import concourse.bass as bass
import concourse.tile as tile
from concourse import bacc, mybir
from concourse.bass_utils import run_bass_kernel_spmd

DIM = 4096
N_HEADS = 32
HEAD_DIM = 128
N_KV_HEADS = 8
BSZ = 4
SEQLEN = 128
T = BSZ * SEQLEN  # 512 tokens
NCORES = 8
HQ = N_HEADS // NCORES  # 4 query heads per core
EQ = HQ * HEAD_DIM  # 512 local q features
ND = DIM // 128  # 32 contraction tiles
SCALE = 1.0 / math.sqrt(HEAD_DIM)

F32 = mybir.dt.float32
F32R = mybir.dt.float32r
BF16 = mybir.dt.bfloat16
AX = mybir.AxisListType
ACTF = mybir.ActivationFunctionType
PSUM = bass.MemorySpace.PSUM

_STATE: dict = {}
LAST_RESULT = None


def _install_ntff_hook():
    """Register the axon NTFF profile hook if the image lacks antenv.axon_hooks.

    Lets run_bass_kernel_spmd(trace=True) return exec_time_ns + perfetto
    under axon. Best-effort: any failure leaves tracing disabled but the
    kernel still runs.
    """
    import os
    import sys
    import types

    try:
        import antenv.axon_hooks  # noqa: F401

        return
    except ImportError:
        pass
    try:
        import antenv
        from trn_agent_boot.trn_boot import _ntff_profile_via_ctypes

        mod = types.ModuleType("antenv.axon_hooks")
        holder = {"hook": None}
        mod.set_axon_ntff_profile_hook = lambda h: holder.__setitem__("hook", h)
        mod.get_axon_ntff_profile_hook = lambda: holder["hook"]
        sys.modules["antenv.axon_hooks"] = mod
        antenv.axon_hooks = mod
        so = "/opt/axon/libaxon_pjrt.so"
        if os.path.exists(so):
            hook = _ntff_profile_via_ctypes(so)
            if hook is not None:
                mod.set_axon_ntff_profile_hook(hook)
    except Exception:
        pass


_install_ntff_hook()


def _build_nc():
    nc = bacc.Bacc(
        "TRN2",
        target_bir_lowering=False,
        debug=False,
        enable_asserts=False,
        num_devices=NCORES,
    )
    xT = nc.dram_tensor("xT", [DIM, T], F32R, kind="ExternalInput").ap()
    wqT = nc.dram_tensor("wqT", [DIM, EQ], F32R, kind="ExternalInput").ap()
    wkT = nc.dram_tensor("wkT", [DIM, HEAD_DIM], F32R, kind="ExternalInput").ap()
    wvT = nc.dram_tensor("wvT", [DIM, HEAD_DIM], F32R, kind="ExternalInput").ap()
    woT = nc.dram_tensor("woT", [EQ, DIM], BF16, kind="ExternalInput").ap()
    mask4 = nc.dram_tensor("mask4", [128, T], F32, kind="ExternalInput").ap()
    cq = nc.dram_tensor("cq", [128, T], F32, kind="ExternalInput").ap()
    sq = nc.dram_tensor("sq", [128, T], F32, kind="ExternalInput").ap()
    ck = nc.dram_tensor("ck", [128, T], F32, kind="ExternalInput").ap()
    sk = nc.dram_tensor("sk", [128, T], F32, kind="ExternalInput").ap()
    ident = nc.dram_tensor("ident", [128, 128], BF16, kind="ExternalInput").ap()
    yT = nc.dram_tensor("yT", [DIM, T], BF16, kind="ExternalOutput").ap()

    with tile.TileContext(nc) as tc, ExitStack() as ctx:
        const = ctx.enter_context(tc.tile_pool(name="const", bufs=1))
        xp = ctx.enter_context(tc.tile_pool(name="xp", bufs=2))
        wqp = ctx.enter_context(tc.tile_pool(name="wqp", bufs=2))
        wkvp = ctx.enter_context(tc.tile_pool(name="wkvp", bufs=2))
        rt = ctx.enter_context(tc.tile_pool(name="rt", bufs=3))
        sm = ctx.enter_context(tc.tile_pool(name="sm", bufs=3))
        yp = ctx.enter_context(tc.tile_pool(name="yp", bufs=4))
        ps = ctx.enter_context(tc.tile_pool(name="ps", bufs=8, space=PSUM))

        qT_sb = const.tile([128, HQ * T], F32, tag="qT")
        kT_sb = const.tile([128, T], F32, tag="kT")
        vT_sb = const.tile([128, T], BF16, tag="vT")
        v_sb = const.tile([128, BSZ * HEAD_DIM], BF16, tag="v")
        oT_sb = const.tile([128, HQ * T], BF16, tag="oT")

        # ---- projection pass: one sweep over the 32 d-tiles ----
        # DMA batching: x/wq in 2MB groups of 8 d-tiles (sync / scalar HWDGE
        # rings), wk/wv in 1MB groups of 16 d-tiles (gpsimd SWDGE) so the
        # three DGE rings run in parallel and transfers are big enough for
        # near-peak HBM rate.
        GD = 8  # d-tiles per x/wq DMA group
        NG = ND // GD
        ps_q = [ps.tile([128, T], F32, tag="ps", name=f"ps_q{h}") for h in range(HQ)]
        ps_k = ps.tile([128, T], F32, tag="ps")
        ps_v = ps.tile([128, T], F32, tag="ps")
        wk_g = []
        for g in range(2):
            rows = slice(g * 16 * 128, (g + 1) * 16 * 128)
            wkt = wkvp.tile([128, 16, HEAD_DIM], F32R, tag="wk", name=f"wk{g}")
            nc.gpsimd.dma_start(wkt[:], wkT[rows, :].rearrange("(j p) e -> p j e", p=128))
            wvt = wkvp.tile([128, 16, HEAD_DIM], F32R, tag="wv", name=f"wv{g}")
            nc.gpsimd.dma_start(wvt[:], wvT[rows, :].rearrange("(j p) e -> p j e", p=128))
            wk_g.append((wkt, wvt))
        for g in range(NG):
            rows = slice(g * GD * 128, (g + 1) * GD * 128)
            xt = xp.tile([128, GD, T], F32R, tag="x")
            nc.sync.dma_start(xt[:], xT[rows, :].rearrange("(j p) t -> p j t", p=128))
            wqt = wqp.tile([128, GD, EQ], F32R, tag="wq")
            nc.scalar.dma_start(wqt[:], wqT[rows, :].rearrange("(j p) e -> p j e", p=128))
            for jj in range(GD):
                j = g * GD + jj
                st, sp = (j == 0), (j == ND - 1)
                xr = xt[:, jj, :]
                for h in range(HQ):
                    nc.tensor.matmul(
                        ps_q[h][:],
                        wqt[:, jj, h * 128 : (h + 1) * 128],
                        xr,
                        start=st,
                        stop=sp,
                    )
                wkt, wvt = wk_g[j // 16]
                nc.tensor.matmul(ps_k[:], wkt[:, j % 16, :], xr, start=st, stop=sp)
                nc.tensor.matmul(ps_v[:], wvt[:, j % 16, :], xr, start=st, stop=sp)

        # constants are only needed from the rope/attention phase on; emit
        # their DMAs after the projection stream so they queue behind it.
        mask_sb = const.tile([128, T], F32, tag="mask")
        nc.scalar.dma_start(mask_sb[:], mask4)
        cq_sb = const.tile([128, T], F32, tag="cq")
        nc.scalar.dma_start(cq_sb[:], cq)
        sq_sb = const.tile([128, T], F32, tag="sq")
        nc.scalar.dma_start(sq_sb[:], sq)
        ck_sb = const.tile([128, T], F32, tag="ck")
        nc.scalar.dma_start(ck_sb[:], ck)
        sk_sb = const.tile([128, T], F32, tag="sk")
        nc.scalar.dma_start(sk_sb[:], sk)
        ident_sb = const.tile([128, 128], BF16, tag="ident")
        nc.scalar.dma_start(ident_sb[:], ident)
        wo_sb = const.tile([128, HQ * DIM], BF16, tag="wo")
        nc.gpsimd.dma_start(
            wo_sb[:].rearrange("p (j c) -> p j c", j=HQ),
            woT.rearrange("(j p) c -> p j c", p=128),
        )

        # ---- RoPE (rotate_half form thanks to host-side row permutation) ----
        def rope(dst_ap, pssrc, ctab, stab):
            swp = rt.tile([128, T], F32, tag="swp")
            nc.scalar.copy(swp[0:64, :], pssrc[64:128, :])
            nc.scalar.copy(swp[64:128, :], pssrc[0:64, :])
            prod = rt.tile([128, T], F32, tag="prod")
            nc.vector.tensor_mul(prod[:], pssrc[:], ctab)
            nc.vector.tensor_mul(swp[:], swp[:], stab)
            nc.vector.tensor_add(dst_ap, prod[:], swp[:])

        rope(kT_sb[:], ps_k[:], ck_sb[:], sk_sb[:])
        # v: evict to bf16, then transpose per batch into [tk, hd]
        nc.scalar.copy(vT_sb[:], ps_v[:])
        for b in range(BSZ):
            bs = slice(b * 128, (b + 1) * 128)
            ps_t = ps.tile([128, T], BF16, tag="ps")
            nc.tensor.transpose(ps_t[:, 0:128], vT_sb[:, bs], ident_sb[:])
            nc.vector.tensor_copy(v_sb[:, bs], ps_t[:, 0:128])
        for h in range(HQ):
            rope(qT_sb[:, h * T : (h + 1) * T], ps_q[h][:], cq_sb[:], sq_sb[:])

        # ---- attention per local head ----
        for h in range(HQ):
            ps_s = ps.tile([128, T], F32, tag="ps")
            for b in range(BSZ):
                bs = slice(b * 128, (b + 1) * 128)
                nc.tensor.matmul(
                    ps_s[:, bs],
                    qT_sb[:, h * T + b * 128 : h * T + (b + 1) * 128],
                    kT_sb[:, bs],
                    start=True,
                    stop=True,
                )
            s_sb = sm.tile([128, T], F32, tag="s")
            nc.vector.tensor_add(s_sb[:], ps_s[:], mask_sb[:])
            nmx = sm.tile([128, BSZ], F32, tag="nmx")
            den = sm.tile([128, BSZ], F32, tag="den")
            rden = sm.tile([128, BSZ], F32, tag="rden")
            p_sb = sm.tile([128, T], BF16, tag="p")
            for b in range(BSZ):
                bs = slice(b * 128, (b + 1) * 128)
                nc.vector.reduce_max(
                    nmx[:, b : b + 1], s_sb[:, bs], axis=AX.X, negate=True
                )
                nc.scalar.activation(
                    p_sb[:, bs],
                    s_sb[:, bs],
                    ACTF.Exp,
                    bias=nmx[:, b : b + 1],
                    accum_out=den[:, b : b + 1],
                )
            nc.vector.reciprocal(rden[:], den[:])
            for b in range(BSZ):
                bs = slice(b * 128, (b + 1) * 128)
                nc.vector.tensor_scalar_mul(p_sb[:, bs], p_sb[:, bs], rden[:, b : b + 1])
                ps_pt = ps.tile([128, T], BF16, tag="ps")
                nc.tensor.transpose(ps_pt[:, 0:128], p_sb[:, bs], ident_sb[:])
                pt_sb = sm.tile([128, 128], BF16, tag="pt")
                nc.scalar.copy(pt_sb[:], ps_pt[:, 0:128])
                ps_o = ps.tile([128, T], F32, tag="ps")
                nc.tensor.matmul(
                    ps_o[:, 0:128], v_sb[:, bs], pt_sb[:], start=True, stop=True
                )
                nc.vector.tensor_copy(
                    oT_sb[:, h * T + b * 128 : h * T + (b + 1) * 128], ps_o[:, 0:128]
                )

        # ---- output projection over local features (partial sums) ----
        for dt in range(ND):
            ps_y = ps.tile([128, T], F32, tag="ps")
            for j in range(HQ):
                nc.tensor.matmul(
                    ps_y[:],
                    wo_sb[:, j * DIM + dt * 128 : j * DIM + (dt + 1) * 128],
                    oT_sb[:, j * T : (j + 1) * T],
                    start=(j == 0),
                    stop=(j == HQ - 1),
                )
            y_sb = yp.tile([128, T], BF16, tag="y")
            if dt % 2 == 0:
                nc.vector.tensor_copy(y_sb[:], ps_y[:])
                nc.sync.dma_start(yT[dt * 128 : (dt + 1) * 128, :], y_sb[:])
            else:
                nc.scalar.copy(y_sb[:], ps_y[:])
                nc.scalar.dma_start(yT[dt * 128 : (dt + 1) * 128, :], y_sb[:])

    nc.compile()
    return nc


def get_nc():
    if "nc" not in _STATE:
        _STATE["nc"] = _build_nc()
    return _STATE["nc"]


def _prep_in_maps(x, wq, wk, wv, wo, freqs_cos, freqs_sin, mask):
    f32 = np.float32
    x = np.asarray(x, f32)
    wq = np.asarray(wq, f32)
    wk = np.asarray(wk, f32)
    wv = np.asarray(wv, f32)
    wo = np.asarray(wo, f32)
    fc = np.asarray(freqs_cos, f32)
    fs = np.asarray(freqs_sin, f32)
    mask = np.asarray(mask, f32)

    # even features first, then odd: (2i, 2i+1) pairs -> (i, i+64)
    perm = np.concatenate([np.arange(0, HEAD_DIM, 2), np.arange(1, HEAD_DIM, 2)])
    wqp = wq.reshape(N_HEADS, HEAD_DIM, DIM)[:, perm, :].reshape(DIM, DIM)
    wkp = wk.reshape(N_KV_HEADS, HEAD_DIM, DIM)[:, perm, :].reshape(
        N_KV_HEADS * HEAD_DIM, DIM
    )

    xT = np.ascontiguousarray(x.reshape(T, DIM).T)
    C0 = np.vstack([fc.T, fc.T])  # [128, 128]: row p -> cos[t, p % 64]
    S0 = np.vstack([-fs.T, fs.T])
    cq = np.ascontiguousarray(np.tile(C0 * SCALE, (1, BSZ)))
    sq = np.ascontiguousarray(np.tile(S0 * SCALE, (1, BSZ)))
    ck = np.ascontiguousarray(np.tile(C0, (1, BSZ)))
    sk = np.ascontiguousarray(np.tile(S0, (1, BSZ)))
    mask4 = np.ascontiguousarray(np.tile(mask[0, 0], (1, BSZ)))
    ident = np.eye(128, dtype=ml_dtypes.bfloat16)

    in_maps = []
    for c in range(NCORES):
        qrows = slice(c * EQ, (c + 1) * EQ)
        krows = slice(c * HEAD_DIM, (c + 1) * HEAD_DIM)
        in_maps.append(
            {
                "xT": xT,
                "wqT": np.ascontiguousarray(wqp[qrows, :].T),
                "wkT": np.ascontiguousarray(wkp[krows, :].T),
                "wvT": np.ascontiguousarray(wv[krows, :].T),
                "woT": np.ascontiguousarray(wo[:, qrows].T).astype(ml_dtypes.bfloat16),
                "mask4": mask4,
                "cq": cq,
                "sq": sq,
                "ck": ck,
                "sk": sk,
                "ident": ident,
            }
        )
    return in_maps


def kernel(
    x,
    wq,
    wk,
    wv,
    wo,
    cache_k,
    cache_v,
    freqs_cos,
    freqs_sin,
    mask,
    start_pos,
    *,
    trace=False,
    trace_kwargs=None,
):
    global LAST_RESULT
    sp = int(np.asarray(start_pos))
    assert sp == 0, f"kernel specialized for start_pos=0, got {sp}"

    in_maps = _prep_in_maps(x, wq, wk, wv, wo, freqs_cos, freqs_sin, mask)
    nc = get_nc()
    res = run_bass_kernel_spmd(
        nc,
        in_maps,
        core_ids=list(range(NCORES)),
        trace=trace,
        **(trace_kwargs or {}),
    )
    LAST_RESULT = res
    acc = np.zeros((DIM, T), np.float32)
    for c in range(NCORES):
        acc += res.results[c]["yT"].astype(np.float32)
    return np.ascontiguousarray(acc.T).reshape(BSZ, SEQLEN, DIM)
